# revision 14
# baseline (speedup 1.0000x reference)
"""CanineEmbeddings (multi-hash bucket embedding lookup + LayerNorm) on 8 TRN2 cores.

Strategy (data-parallel over batch):
  - Each of the 8 cores handles one batch row (8192 tokens).
  - Hash tables (8 x 16384 x 96 f32) are padded host-side to 128 floats/row
    (512B, zero pad) and replicated to every core in DRAM.
  - Tables are processed in PAIRS so a gathered row index
    (pair_half * 16384 + hash) fits in int16 (max 32767) -> 4 dma_gather
    calls per 1024-token group instead of 8.
  - Hashes ((id+1)*prime & 16383) are computed on-device with DVE int ops.
  - dma_gather (SWDGE) fetches 512B rows; gathered layout is
    [128 tokens(part), pair, half, chunk, 128f].
  - LayerNorm per token via bn_stats/bn_aggr on DVE (exact: pad columns are
    excluded via strided 0:96 access patterns), apply via ACT
    Identity(x*rstd - mean*rstd), which also packs 8x96 shards into
    contiguous 768-float rows.
  - Packed rows are DMA'd straight to the output (3072B descriptors).
"""

import contextlib
import ctypes
import os
import sys
import types

import numpy as np

import concourse.bacc as bacc
import concourse.bass as bass
import concourse.mybir as mybir
import concourse.tile as tile
from concourse.bass_utils import run_bass_kernel_spmd
from concourse.library_config import mlp as _mlp_lib
from concourse.tile import add_dep_helper


def _ensure_axon_ntff_hook():
    """The agent image's ``antenv`` lacks ``axon_hooks``; provide it (and the
    ctypes NTFF profile hook) so run_bass_kernel_spmd(trace=True) works.
    Degrades to a None hook (no trace, run still works) on any failure."""
    if "antenv.axon_hooks" in sys.modules:
        return
    hook = None
    try:
        so_path = "/opt/axon/libaxon_pjrt.so"
        lib = ctypes.CDLL(so_path)
        if hasattr(lib, "axon_start_nrt_profile"):
            lib.axon_start_nrt_profile.argtypes = [
                ctypes.POINTER(ctypes.c_int64),
                ctypes.c_size_t,
            ]
            lib.axon_start_nrt_profile.restype = ctypes.c_int64
            lib.axon_stop_nrt_profile.argtypes = [ctypes.c_char_p]
            lib.axon_stop_nrt_profile.restype = ctypes.c_int64

            @contextlib.contextmanager
            def _hook(output_dir, device_ids):
                import jax

                jax.devices()
                if device_ids:
                    ids = (ctypes.c_int64 * len(device_ids))(*device_ids)
                    rc = lib.axon_start_nrt_profile(ids, len(device_ids))
                else:
                    rc = lib.axon_start_nrt_profile(None, 0)
                if rc != 0:
                    raise RuntimeError(f"axon_start_nrt_profile rc={rc}")
                try:
                    yield
                finally:
                    n = lib.axon_stop_nrt_profile(str(output_dir).encode())
                    print(f"ntff profile: {n} file(s) -> {output_dir}", file=sys.stderr)

            hook = _hook
    except Exception as e:  # pragma: no cover
        print(f"ntff hook unavailable: {e}", file=sys.stderr)
    mod = types.ModuleType("antenv.axon_hooks")
    mod.get_axon_ntff_profile_hook = lambda: hook
    mod.set_axon_ntff_profile_hook = lambda h: None
    sys.modules["antenv.axon_hooks"] = mod


_ensure_axon_ntff_hook()

PRIMES = [31, 43, 59, 61, 73, 97, 103, 113]
NUM_HASHES = 8
NUM_BUCKETS = 16384
HIDDEN = 768
SHARD = 96
PAD = 128  # padded floats per table row (512 bytes)
LN_EPS = 1e-6
N_CORES = 8
GROUP = 1024  # tokens per gather group
CHUNK = 128  # tokens per LayerNorm chunk (one partition sweep)
N_PAIRS = NUM_HASHES // 2

AluOp = mybir.AluOpType
Act = mybir.ActivationFunctionType


def _build(tok_per_core: int, affine: bool, enable_asserts: bool = False):
    n_groups = tok_per_core // GROUP
    n_chunks = GROUP // CHUNK  # 8
    wrap_s = GROUP // 16  # 64
    f32, i32, i16 = mybir.dt.float32, mybir.dt.int32, mybir.dt.int16

    nc = bacc.Bacc(
        "TRN2",
        target_bir_lowering=False,
        debug=False,
        enable_asserts=enable_asserts,
    )

    ids_d = nc.dram_tensor("ids", [128, n_groups * wrap_s], i32, kind="ExternalInput")
    tab_d = nc.dram_tensor(
        "tables", [NUM_HASHES * NUM_BUCKETS, PAD], f32, kind="ExternalInput"
    )
    out_d = nc.dram_tensor("out", [tok_per_core, HIDDEN], f32, kind="ExternalOutput")
    if affine:
        sc_d = nc.dram_tensor("ln_scale", [128, HIDDEN], f32, kind="ExternalInput")
        bi_d = nc.dram_tensor("ln_bias", [128, HIDDEN], f32, kind="ExternalInput")

    from contextlib import ExitStack

    with tile.TileContext(nc) as tc, ExitStack() as ctx:
        const = ctx.enter_context(tc.tile_pool(name="const", bufs=1))
        gpool = ctx.enter_context(tc.tile_pool(name="gather", bufs=2))
        ppool = ctx.enter_context(tc.tile_pool(name="packed", bufs=2))
        spool = ctx.enter_context(tc.tile_pool(name="stats", bufs=8))
        tpool = ctx.enter_context(tc.tile_pool(name="tmp", bufs=2))

        # dma_gather is a Q7 extended instruction living in the 'mlp' ucode
        # library; it must be loaded on the Pool engine before any gather.
        lib_inst = nc.gpsimd.load_library(_mlp_lib).ins

        eps_sb = const.tile([128, 1], f32)
        nc.vector.memset(eps_sb[:], LN_EPS)

        ids_sb = const.tile([128, n_groups, wrap_s], i32)
        nc.sync.dma_start(
            out=ids_sb[:],
            in_=ids_d[:].rearrange("p (g s) -> p g s", g=n_groups),
        )
        if affine:
            sc_sb = const.tile([128, HIDDEN], f32)
            nc.sync.dma_start(out=sc_sb[:], in_=sc_d[:])
            bi_sb = const.tile([128, HIDDEN], f32)
            nc.sync.dma_start(out=bi_sb[:], in_=bi_d[:])

        # idx_all[p, g, pair, half, s] = half*16384 + hash_{2*pair+half}(ids[g, s*16+p%16])
        # DVE arithmetic runs in fp32, so keep every intermediate < 2^24:
        # m = (ids & 16383) + 1 <= 16384; t = m*prime <= 1.85e6; idx = (t & 16383) + half*16384.
        idx_all = const.tile([128, n_groups, N_PAIRS, 2, wrap_s], i16)
        m_sb = const.tile([128, n_groups, wrap_s], i32)
        nc.vector.tensor_scalar(
            out=m_sb[:],
            in0=ids_sb[:],
            scalar1=NUM_BUCKETS - 1,
            scalar2=None,
            op0=AluOp.bitwise_and,
        )
        for h in range(NUM_HASHES):
            # (m+1)*p = m*p + p (all-arith); then (t & 16383) | half*16384
            # (all-bitwise) -- walrus rejects mixed bitwise/arith chains.
            tmp = tpool.tile([128, n_groups, wrap_s], i32)
            nc.vector.tensor_scalar(
                out=tmp[:],
                in0=m_sb[:],
                scalar1=PRIMES[h],
                scalar2=PRIMES[h],
                op0=AluOp.mult,
                op1=AluOp.add,
            )
            tmp2 = tpool.tile([128, n_groups, wrap_s], i32)
            nc.vector.tensor_scalar(
                out=tmp2[:],
                in0=tmp[:],
                scalar1=NUM_BUCKETS - 1,
                scalar2=None,
                op0=AluOp.bitwise_and,
            )
            nc.vector.tensor_copy(
                out=idx_all[:, :, h // 2, h % 2, :], in_=tmp2[:]
            )

        for g in range(n_groups):
            # gathered rows: gt[p, pair, half, chunk, f]
            gt = gpool.tile([128, N_PAIRS, 2, n_chunks, PAD], f32)
            # one gather per hash table; num_idxs is capped by the SWDGE
            # descriptor ring (dynamic_dma_scratch_size/16 = 1024 descs)
            for h in range(NUM_HASHES):
                gi = nc.gpsimd.dma_gather(
                    out_ap=gt[:, h // 2, h % 2],
                    in_ap=tab_d[h * NUM_BUCKETS : (h + 1) * NUM_BUCKETS, :],
                    idxs_ap=idx_all[:, g, h // 2, h % 2, :],
                    num_idxs=GROUP,
                    num_idxs_reg=GROUP,
                    elem_size=PAD,
                )
                add_dep_helper(gi.ins, lib_inst, sync=False, reason="needs mlp lib")
            pk = ppool.tile([128, n_chunks, HIDDEN], f32)
            for c in range(n_chunks):
                stats = spool.tile([128, NUM_HASHES, 6], f32)
                for h in range(NUM_HASHES):
                    nc.vector.bn_stats(
                        out=stats[:, h, :], in_=gt[:, h // 2, h % 2, c, 0:SHARD]
                    )
                mv = spool.tile([128, 2], f32)
                nc.vector.bn_aggr(out=mv[:], in_=stats[:])
                sd = spool.tile([128, 1], f32)
                nc.scalar.activation(
                    out=sd[:], in_=mv[:, 1:2], func=Act.Sqrt, bias=eps_sb[:]
                )
                rstd = spool.tile([128, 1], f32)
                nc.vector.reciprocal(out=rstd[:], in_=sd[:])
                beta = spool.tile([128, 1], f32)
                nc.vector.tensor_scalar(
                    out=beta[:],
                    in0=mv[:, 0:1],
                    scalar1=rstd[:],
                    scalar2=-1.0,
                    op0=AluOp.mult,
                    op1=AluOp.mult,
                )
                # normalize + pack: pk[p, c, h*96+f] = gt[p, h//2, h%2, c, f]*rstd - mean*rstd
                nc.scalar.activation(
                    out=pk[:, c].rearrange("p (a b f) -> p a b f", a=N_PAIRS, b=2),
                    in_=gt[:, :, :, c, 0:SHARD],
                    func=Act.Identity,
                    bias=beta[:],
                    scale=rstd[:],
                )
                if affine:
                    nc.vector.tensor_mul(pk[:, c], pk[:, c], sc_sb[:])
                    nc.vector.tensor_add(pk[:, c], pk[:, c], bi_sb[:])
            dst = bass.AP(
                out_d,
                g * GROUP * HIDDEN,
                [[HIDDEN, CHUNK], [CHUNK * HIDDEN, n_chunks], [1, HIDDEN]],
            )
            nc.sync.dma_start(out=dst, in_=pk[:])

    nc.compile()
    return nc


_kernel_cache: dict = {}
last_results = None


def _get_nc(tok_per_core: int, affine: bool):
    key = (tok_per_core, affine)
    if key not in _kernel_cache:
        _kernel_cache[key] = _build(tok_per_core, affine)
    return _kernel_cache[key]


def _prep_inputs(input_ids, tables, ln_scale, ln_bias):
    input_ids = np.asarray(input_ids)
    tables = np.asarray(tables, dtype=np.float32)
    ln_scale = np.asarray(ln_scale, dtype=np.float32)
    ln_bias = np.asarray(ln_bias, dtype=np.float32)
    B, S = input_ids.shape
    tok_per_core = B * S // N_CORES
    affine = not (np.all(ln_scale == 1.0) and np.all(ln_bias == 0.0))

    tabp = np.zeros((NUM_HASHES, NUM_BUCKETS, PAD), np.float32)
    tabp[:, :, :SHARD] = tables
    tabp = np.ascontiguousarray(tabp.reshape(NUM_HASHES * NUM_BUCKETS, PAD))

    ids_flat = input_ids.reshape(-1).astype(np.int64).astype(np.int32)
    in_maps = []
    for c in range(N_CORES):
        idc = ids_flat[c * tok_per_core : (c + 1) * tok_per_core]
        # wrapped-16 layout: w16[p, g, s] = idc[g*GROUP + s*16 + p], replicated
        # over the 8 gpsimd-core partition groups
        w16 = idc.reshape(-1, GROUP // 16, 16).transpose(2, 0, 1)  # [16, g, s]
        w = np.tile(w16, (8, 1, 1)).reshape(128, -1)
        m = {"ids": np.ascontiguousarray(w), "tables": tabp}
        if affine:
            m["ln_scale"] = np.ascontiguousarray(
                np.broadcast_to(ln_scale[None], (128, HIDDEN))
            )
            m["ln_bias"] = np.ascontiguousarray(
                np.broadcast_to(ln_bias[None], (128, HIDDEN))
            )
        in_maps.append(m)
    return in_maps, tok_per_core, affine, (B, S)


def kernel(input_ids, tables, ln_scale, ln_bias):
    global last_results
    in_maps, tok_per_core, affine, (B, S) = _prep_inputs(
        input_ids, tables, ln_scale, ln_bias
    )
    nc = _get_nc(tok_per_core, affine)
    res = run_bass_kernel_spmd(nc, in_maps, core_ids=list(range(N_CORES)))
    last_results = res
    out = np.stack([r["out"] for r in res.results], axis=0)
    return out.reshape(B, S, HIDDEN)


# revision 16
# speedup vs baseline: 2.4886x; 2.4886x over previous
"""CanineEmbeddings (multi-hash bucket embedding lookup + LayerNorm) on 8 TRN2 cores.

Strategy (data-parallel over batch):
  - Each of the 8 cores handles one batch row (8192 tokens).
  - Hash tables (8 x 16384 x 96 f32) are padded host-side to 128 floats/row
    (512B, zero pad) and replicated to every core in DRAM.
  - Tables are processed in PAIRS so a gathered row index
    (pair_half * 16384 + hash) fits in int16 (max 32767) -> 4 dma_gather
    calls per 1024-token group instead of 8.
  - Hashes ((id+1)*prime & 16383) are computed on-device with DVE int ops.
  - dma_gather (SWDGE) fetches 512B rows; gathered layout is
    [128 tokens(part), pair, half, chunk, 128f].
  - LayerNorm per token via bn_stats/bn_aggr on DVE (exact: pad columns are
    excluded via strided 0:96 access patterns), apply via ACT
    Identity(x*rstd - mean*rstd), which also packs 8x96 shards into
    contiguous 768-float rows.
  - Packed rows are DMA'd straight to the output (3072B descriptors).
"""

import contextlib
import ctypes
import os
import sys
import types

import numpy as np

import concourse.bacc as bacc
import concourse.bass as bass
import concourse.mybir as mybir
import concourse.tile as tile
from concourse.bass_utils import run_bass_kernel_spmd
from concourse.library_config import mlp as _mlp_lib
from concourse.tile import add_dep_helper


def _ensure_axon_ntff_hook():
    """The agent image's ``antenv`` lacks ``axon_hooks``; provide it (and the
    ctypes NTFF profile hook) so run_bass_kernel_spmd(trace=True) works.
    Degrades to a None hook (no trace, run still works) on any failure."""
    if "antenv.axon_hooks" in sys.modules:
        return
    hook = None
    try:
        so_path = "/opt/axon/libaxon_pjrt.so"
        lib = ctypes.CDLL(so_path)
        if hasattr(lib, "axon_start_nrt_profile"):
            lib.axon_start_nrt_profile.argtypes = [
                ctypes.POINTER(ctypes.c_int64),
                ctypes.c_size_t,
            ]
            lib.axon_start_nrt_profile.restype = ctypes.c_int64
            lib.axon_stop_nrt_profile.argtypes = [ctypes.c_char_p]
            lib.axon_stop_nrt_profile.restype = ctypes.c_int64

            @contextlib.contextmanager
            def _hook(output_dir, device_ids):
                import jax

                jax.devices()
                if device_ids:
                    ids = (ctypes.c_int64 * len(device_ids))(*device_ids)
                    rc = lib.axon_start_nrt_profile(ids, len(device_ids))
                else:
                    rc = lib.axon_start_nrt_profile(None, 0)
                if rc != 0:
                    raise RuntimeError(f"axon_start_nrt_profile rc={rc}")
                try:
                    yield
                finally:
                    n = lib.axon_stop_nrt_profile(str(output_dir).encode())
                    print(f"ntff profile: {n} file(s) -> {output_dir}", file=sys.stderr)

            hook = _hook
    except Exception as e:  # pragma: no cover
        print(f"ntff hook unavailable: {e}", file=sys.stderr)
    mod = types.ModuleType("antenv.axon_hooks")
    mod.get_axon_ntff_profile_hook = lambda: hook
    mod.set_axon_ntff_profile_hook = lambda h: None
    sys.modules["antenv.axon_hooks"] = mod


_ensure_axon_ntff_hook()

PRIMES = [31, 43, 59, 61, 73, 97, 103, 113]
NUM_HASHES = 8
NUM_BUCKETS = 16384
HIDDEN = 768
SHARD = 96
PAD = 128  # padded floats per table row (512 bytes)
LN_EPS = 1e-6
N_CORES = 8
GROUP = 1024  # tokens per gather group
CHUNK = 128  # tokens per LayerNorm chunk (one partition sweep)
N_PAIRS = NUM_HASHES // 2

AluOp = mybir.AluOpType
Act = mybir.ActivationFunctionType


def _build(tok_per_core: int, affine: bool, enable_asserts: bool = False):
    n_groups = tok_per_core // GROUP
    n_chunks = GROUP // CHUNK  # 8
    wrap_s = GROUP // 16  # 64
    f32, i32, i16 = mybir.dt.float32, mybir.dt.int32, mybir.dt.int16

    nc = bacc.Bacc(
        "TRN2",
        target_bir_lowering=False,
        debug=False,
        enable_asserts=enable_asserts,
        # dma_gather desc-gen runs on the Q7 cpu pair selected by queue_num;
        # 4 queues let up to 4 gathers generate descriptors concurrently.
        num_swdge_queues=4,
    )

    ids_d = nc.dram_tensor("ids", [128, n_groups * wrap_s], i32, kind="ExternalInput")
    tab_d = nc.dram_tensor(
        "tables", [NUM_HASHES * NUM_BUCKETS, PAD], f32, kind="ExternalInput"
    )
    out_d = nc.dram_tensor("out", [tok_per_core, HIDDEN], f32, kind="ExternalOutput")
    if affine:
        sc_d = nc.dram_tensor("ln_scale", [128, HIDDEN], f32, kind="ExternalInput")
        bi_d = nc.dram_tensor("ln_bias", [128, HIDDEN], f32, kind="ExternalInput")

    from contextlib import ExitStack

    with tile.TileContext(nc) as tc, ExitStack() as ctx:
        const = ctx.enter_context(tc.tile_pool(name="const", bufs=1))
        gpool = ctx.enter_context(tc.tile_pool(name="gather", bufs=2))
        ppool = ctx.enter_context(tc.tile_pool(name="packed", bufs=2))
        spool = ctx.enter_context(tc.tile_pool(name="stats", bufs=8))
        tpool = ctx.enter_context(tc.tile_pool(name="tmp", bufs=2))

        # dma_gather is a Q7 extended instruction living in the 'mlp' ucode
        # library; it must be loaded on the Pool engine before any gather.
        lib_inst = nc.gpsimd.load_library(_mlp_lib).ins

        eps_sb = const.tile([128, 1], f32)
        nc.vector.memset(eps_sb[:], LN_EPS)

        ids_sb = const.tile([128, n_groups, wrap_s], i32)
        nc.sync.dma_start(
            out=ids_sb[:],
            in_=ids_d[:].rearrange("p (g s) -> p g s", g=n_groups),
        )
        if affine:
            sc_sb = const.tile([128, HIDDEN], f32)
            nc.sync.dma_start(out=sc_sb[:], in_=sc_d[:])
            bi_sb = const.tile([128, HIDDEN], f32)
            nc.sync.dma_start(out=bi_sb[:], in_=bi_d[:])

        # idx_all[p, g, pair, half, s] = half*16384 + hash_{2*pair+half}(ids[g, s*16+p%16])
        # DVE arithmetic runs in fp32, so keep every intermediate < 2^24:
        # m = (ids & 16383) + 1 <= 16384; t = m*prime <= 1.85e6; idx = (t & 16383) + half*16384.
        idx_all = const.tile([128, n_groups, N_PAIRS, 2, wrap_s], i16)
        m_sb = const.tile([128, n_groups, wrap_s], i32)
        nc.vector.tensor_scalar(
            out=m_sb[:],
            in0=ids_sb[:],
            scalar1=NUM_BUCKETS - 1,
            scalar2=None,
            op0=AluOp.bitwise_and,
        )
        for h in range(NUM_HASHES):
            # (m+1)*p = m*p + p (all-arith); then (t & 16383) | half*16384
            # (all-bitwise) -- walrus rejects mixed bitwise/arith chains.
            tmp = tpool.tile([128, n_groups, wrap_s], i32)
            nc.vector.tensor_scalar(
                out=tmp[:],
                in0=m_sb[:],
                scalar1=PRIMES[h],
                scalar2=PRIMES[h],
                op0=AluOp.mult,
                op1=AluOp.add,
            )
            tmp2 = tpool.tile([128, n_groups, wrap_s], i32)
            nc.vector.tensor_scalar(
                out=tmp2[:],
                in0=tmp[:],
                scalar1=NUM_BUCKETS - 1,
                scalar2=None,
                op0=AluOp.bitwise_and,
            )
            nc.vector.tensor_copy(
                out=idx_all[:, :, h // 2, h % 2, :], in_=tmp2[:]
            )

        for g in range(n_groups):
            # gathered rows: gt[p, pair, half, chunk, f]
            gt = gpool.tile([128, N_PAIRS, 2, n_chunks, PAD], f32)
            # one gather per hash table; num_idxs is capped by the SWDGE
            # descriptor ring (dynamic_dma_scratch_size/16 = 1024 descs)
            for h in range(NUM_HASHES):
                gi = nc.gpsimd.dma_gather(
                    out_ap=gt[:, h // 2, h % 2],
                    in_ap=tab_d[h * NUM_BUCKETS : (h + 1) * NUM_BUCKETS, :],
                    idxs_ap=idx_all[:, g, h // 2, h % 2, :],
                    num_idxs=GROUP,
                    num_idxs_reg=GROUP,
                    elem_size=PAD,
                    queue_num=h % 4,
                )
                add_dep_helper(gi.ins, lib_inst, sync=False, reason="needs mlp lib")
            pk = ppool.tile([128, n_chunks, HIDDEN], f32)
            for c in range(n_chunks):
                stats = spool.tile([128, NUM_HASHES, 6], f32)
                for h in range(NUM_HASHES):
                    nc.vector.bn_stats(
                        out=stats[:, h, :], in_=gt[:, h // 2, h % 2, c, 0:SHARD]
                    )
                mv = spool.tile([128, 2], f32)
                nc.vector.bn_aggr(out=mv[:], in_=stats[:])
                sd = spool.tile([128, 1], f32)
                nc.scalar.activation(
                    out=sd[:], in_=mv[:, 1:2], func=Act.Sqrt, bias=eps_sb[:]
                )
                rstd = spool.tile([128, 1], f32)
                nc.vector.reciprocal(out=rstd[:], in_=sd[:])
                beta = spool.tile([128, 1], f32)
                nc.vector.tensor_scalar(
                    out=beta[:],
                    in0=mv[:, 0:1],
                    scalar1=rstd[:],
                    scalar2=-1.0,
                    op0=AluOp.mult,
                    op1=AluOp.mult,
                )
                # normalize + pack: pk[p, c, h*96+f] = gt[p, h//2, h%2, c, f]*rstd - mean*rstd
                nc.scalar.activation(
                    out=pk[:, c].rearrange("p (a b f) -> p a b f", a=N_PAIRS, b=2),
                    in_=gt[:, :, :, c, 0:SHARD],
                    func=Act.Identity,
                    bias=beta[:],
                    scale=rstd[:],
                )
                if affine:
                    nc.vector.tensor_mul(pk[:, c], pk[:, c], sc_sb[:])
                    nc.vector.tensor_add(pk[:, c], pk[:, c], bi_sb[:])
            dst = bass.AP(
                out_d,
                g * GROUP * HIDDEN,
                [[HIDDEN, CHUNK], [CHUNK * HIDDEN, n_chunks], [1, HIDDEN]],
            )
            nc.sync.dma_start(out=dst, in_=pk[:])

    nc.compile()
    return nc


_kernel_cache: dict = {}
last_results = None


def _get_nc(tok_per_core: int, affine: bool):
    key = (tok_per_core, affine)
    if key not in _kernel_cache:
        _kernel_cache[key] = _build(tok_per_core, affine)
    return _kernel_cache[key]


def _prep_inputs(input_ids, tables, ln_scale, ln_bias):
    input_ids = np.asarray(input_ids)
    tables = np.asarray(tables, dtype=np.float32)
    ln_scale = np.asarray(ln_scale, dtype=np.float32)
    ln_bias = np.asarray(ln_bias, dtype=np.float32)
    B, S = input_ids.shape
    tok_per_core = B * S // N_CORES
    affine = not (np.all(ln_scale == 1.0) and np.all(ln_bias == 0.0))

    tabp = np.zeros((NUM_HASHES, NUM_BUCKETS, PAD), np.float32)
    tabp[:, :, :SHARD] = tables
    tabp = np.ascontiguousarray(tabp.reshape(NUM_HASHES * NUM_BUCKETS, PAD))

    ids_flat = input_ids.reshape(-1).astype(np.int64).astype(np.int32)
    in_maps = []
    for c in range(N_CORES):
        idc = ids_flat[c * tok_per_core : (c + 1) * tok_per_core]
        # wrapped-16 layout: w16[p, g, s] = idc[g*GROUP + s*16 + p], replicated
        # over the 8 gpsimd-core partition groups
        w16 = idc.reshape(-1, GROUP // 16, 16).transpose(2, 0, 1)  # [16, g, s]
        w = np.tile(w16, (8, 1, 1)).reshape(128, -1)
        m = {"ids": np.ascontiguousarray(w), "tables": tabp}
        if affine:
            m["ln_scale"] = np.ascontiguousarray(
                np.broadcast_to(ln_scale[None], (128, HIDDEN))
            )
            m["ln_bias"] = np.ascontiguousarray(
                np.broadcast_to(ln_bias[None], (128, HIDDEN))
            )
        in_maps.append(m)
    return in_maps, tok_per_core, affine, (B, S)


def kernel(input_ids, tables, ln_scale, ln_bias):
    global last_results
    in_maps, tok_per_core, affine, (B, S) = _prep_inputs(
        input_ids, tables, ln_scale, ln_bias
    )
    nc = _get_nc(tok_per_core, affine)
    res = run_bass_kernel_spmd(nc, in_maps, core_ids=list(range(N_CORES)))
    last_results = res
    out = np.stack([r["out"] for r in res.results], axis=0)
    return out.reshape(B, S, HIDDEN)


# revision 18
# speedup vs baseline: 2.6281x; 1.0560x over previous
"""CanineEmbeddings (multi-hash bucket embedding lookup + LayerNorm) on 8 TRN2 cores.

Strategy (data-parallel over batch):
  - Each of the 8 cores handles one batch row (8192 tokens).
  - Hash tables (8 x 16384 x 96 f32) are padded host-side to 128 floats/row
    (512B, zero pad) and replicated to every core in DRAM.
  - Tables are processed in PAIRS so a gathered row index
    (pair_half * 16384 + hash) fits in int16 (max 32767) -> 4 dma_gather
    calls per 1024-token group instead of 8.
  - Hashes ((id+1)*prime & 16383) are computed on-device with DVE int ops.
  - dma_gather (SWDGE) fetches 512B rows; gathered layout is
    [128 tokens(part), pair, half, chunk, 128f].
  - LayerNorm per token via bn_stats/bn_aggr on DVE (exact: pad columns are
    excluded via strided 0:96 access patterns), apply via ACT
    Identity(x*rstd - mean*rstd), which also packs 8x96 shards into
    contiguous 768-float rows.
  - Packed rows are DMA'd straight to the output (3072B descriptors).
"""

import contextlib
import ctypes
import os
import sys
import types

import numpy as np

import concourse.bacc as bacc
import concourse.bass as bass
import concourse.mybir as mybir
import concourse.tile as tile
from concourse.bass_utils import run_bass_kernel_spmd
from concourse.library_config import mlp as _mlp_lib
from concourse.tile import add_dep_helper


def _ensure_axon_ntff_hook():
    """The agent image's ``antenv`` lacks ``axon_hooks``; provide it (and the
    ctypes NTFF profile hook) so run_bass_kernel_spmd(trace=True) works.
    Degrades to a None hook (no trace, run still works) on any failure."""
    if "antenv.axon_hooks" in sys.modules:
        return
    hook = None
    try:
        so_path = "/opt/axon/libaxon_pjrt.so"
        lib = ctypes.CDLL(so_path)
        if hasattr(lib, "axon_start_nrt_profile"):
            lib.axon_start_nrt_profile.argtypes = [
                ctypes.POINTER(ctypes.c_int64),
                ctypes.c_size_t,
            ]
            lib.axon_start_nrt_profile.restype = ctypes.c_int64
            lib.axon_stop_nrt_profile.argtypes = [ctypes.c_char_p]
            lib.axon_stop_nrt_profile.restype = ctypes.c_int64

            @contextlib.contextmanager
            def _hook(output_dir, device_ids):
                import jax

                jax.devices()
                if device_ids:
                    ids = (ctypes.c_int64 * len(device_ids))(*device_ids)
                    rc = lib.axon_start_nrt_profile(ids, len(device_ids))
                else:
                    rc = lib.axon_start_nrt_profile(None, 0)
                if rc != 0:
                    raise RuntimeError(f"axon_start_nrt_profile rc={rc}")
                try:
                    yield
                finally:
                    n = lib.axon_stop_nrt_profile(str(output_dir).encode())
                    print(f"ntff profile: {n} file(s) -> {output_dir}", file=sys.stderr)

            hook = _hook
    except Exception as e:  # pragma: no cover
        print(f"ntff hook unavailable: {e}", file=sys.stderr)
    mod = types.ModuleType("antenv.axon_hooks")
    mod.get_axon_ntff_profile_hook = lambda: hook
    mod.set_axon_ntff_profile_hook = lambda h: None
    sys.modules["antenv.axon_hooks"] = mod


_ensure_axon_ntff_hook()

PRIMES = [31, 43, 59, 61, 73, 97, 103, 113]
NUM_HASHES = 8
NUM_BUCKETS = 16384
HIDDEN = 768
SHARD = 96
PAD = 128  # padded floats per table row (512 bytes)
LN_EPS = 1e-6
N_CORES = 8
GROUP = 1024  # tokens per gather group
CHUNK = 128  # tokens per LayerNorm chunk (one partition sweep)
N_PAIRS = NUM_HASHES // 2

AluOp = mybir.AluOpType
Act = mybir.ActivationFunctionType


def _build(tok_per_core: int, affine: bool, enable_asserts: bool = False):
    n_groups = tok_per_core // GROUP
    n_chunks = GROUP // CHUNK  # 8
    wrap_s = GROUP // 16  # 64
    f32, i32, i16 = mybir.dt.float32, mybir.dt.int32, mybir.dt.int16

    nc = bacc.Bacc(
        "TRN2",
        target_bir_lowering=False,
        debug=False,
        enable_asserts=enable_asserts,
        # dma_gather desc-gen runs on the Q7 cpu pair selected by queue_num;
        # 4 queues let up to 4 gathers generate descriptors concurrently.
        num_swdge_queues=4,
    )

    ids_d = nc.dram_tensor("ids", [128, n_groups * wrap_s], i32, kind="ExternalInput")
    tab_d = nc.dram_tensor(
        "tables", [NUM_HASHES * NUM_BUCKETS, PAD], f32, kind="ExternalInput"
    )
    out_d = nc.dram_tensor("out", [tok_per_core, HIDDEN], f32, kind="ExternalOutput")
    if affine:
        sc_d = nc.dram_tensor("ln_scale", [128, HIDDEN], f32, kind="ExternalInput")
        bi_d = nc.dram_tensor("ln_bias", [128, HIDDEN], f32, kind="ExternalInput")

    from contextlib import ExitStack

    with tile.TileContext(nc) as tc, ExitStack() as ctx:
        const = ctx.enter_context(tc.tile_pool(name="const", bufs=1))
        gpool = ctx.enter_context(tc.tile_pool(name="gather", bufs=3))
        ppool = ctx.enter_context(tc.tile_pool(name="packed", bufs=2))
        spool = ctx.enter_context(tc.tile_pool(name="stats", bufs=8))
        tpool = ctx.enter_context(tc.tile_pool(name="tmp", bufs=2))

        # dma_gather is a Q7 extended instruction living in the 'mlp' ucode
        # library; it must be loaded on the Pool engine before any gather.
        lib_inst = nc.gpsimd.load_library(_mlp_lib).ins

        eps_sb = const.tile([128, 1], f32)
        nc.vector.memset(eps_sb[:], LN_EPS)

        ids_sb = const.tile([128, n_groups, wrap_s], i32)
        nc.sync.dma_start(
            out=ids_sb[:],
            in_=ids_d[:].rearrange("p (g s) -> p g s", g=n_groups),
        )
        if affine:
            sc_sb = const.tile([128, HIDDEN], f32)
            nc.sync.dma_start(out=sc_sb[:], in_=sc_d[:])
            bi_sb = const.tile([128, HIDDEN], f32)
            nc.sync.dma_start(out=bi_sb[:], in_=bi_d[:])

        # idx_all[p, g, pair, half, s] = half*16384 + hash_{2*pair+half}(ids[g, s*16+p%16])
        # DVE arithmetic runs in fp32, so keep every intermediate < 2^24:
        # m = (ids & 16383) + 1 <= 16384; t = m*prime <= 1.85e6; idx = (t & 16383) + half*16384.
        idx_all = const.tile([128, n_groups, N_PAIRS, 2, wrap_s], i16)
        m_sb = const.tile([128, n_groups, wrap_s], i32)
        nc.vector.tensor_scalar(
            out=m_sb[:],
            in0=ids_sb[:],
            scalar1=NUM_BUCKETS - 1,
            scalar2=None,
            op0=AluOp.bitwise_and,
        )
        for h in range(NUM_HASHES):
            # (m+1)*p = m*p + p (all-arith); then (t & 16383) | half*16384
            # (all-bitwise) -- walrus rejects mixed bitwise/arith chains.
            tmp = tpool.tile([128, n_groups, wrap_s], i32)
            nc.vector.tensor_scalar(
                out=tmp[:],
                in0=m_sb[:],
                scalar1=PRIMES[h],
                scalar2=PRIMES[h],
                op0=AluOp.mult,
                op1=AluOp.add,
            )
            tmp2 = tpool.tile([128, n_groups, wrap_s], i32)
            nc.vector.tensor_scalar(
                out=tmp2[:],
                in0=tmp[:],
                scalar1=NUM_BUCKETS - 1,
                scalar2=None,
                op0=AluOp.bitwise_and,
            )
            nc.vector.tensor_copy(
                out=idx_all[:, :, h // 2, h % 2, :], in_=tmp2[:]
            )

        for g in range(n_groups):
            # gathered rows: gt[p, pair, half, chunk, f]
            gt = gpool.tile([128, N_PAIRS, 2, n_chunks, PAD], f32)
            # one gather per hash table; num_idxs is capped by the SWDGE
            # descriptor ring (dynamic_dma_scratch_size/16 = 1024 descs)
            for h in range(NUM_HASHES):
                gi = nc.gpsimd.dma_gather(
                    out_ap=gt[:, h // 2, h % 2],
                    in_ap=tab_d[h * NUM_BUCKETS : (h + 1) * NUM_BUCKETS, :],
                    idxs_ap=idx_all[:, g, h // 2, h % 2, :],
                    num_idxs=GROUP,
                    num_idxs_reg=GROUP,
                    elem_size=PAD,
                    queue_num=h % 4,
                )
                add_dep_helper(gi.ins, lib_inst, sync=False, reason="needs mlp lib")
            pk = ppool.tile([128, n_chunks, HIDDEN], f32)
            for c in range(n_chunks):
                stats = spool.tile([128, NUM_HASHES, 6], f32)
                for h in range(NUM_HASHES):
                    nc.vector.bn_stats(
                        out=stats[:, h, :], in_=gt[:, h // 2, h % 2, c, 0:SHARD]
                    )
                mv = spool.tile([128, 2], f32)
                nc.vector.bn_aggr(out=mv[:], in_=stats[:])
                sd = spool.tile([128, 1], f32)
                nc.scalar.activation(
                    out=sd[:], in_=mv[:, 1:2], func=Act.Sqrt, bias=eps_sb[:]
                )
                rstd = spool.tile([128, 1], f32)
                nc.vector.reciprocal(out=rstd[:], in_=sd[:])
                beta = spool.tile([128, 1], f32)
                nc.vector.tensor_scalar(
                    out=beta[:],
                    in0=mv[:, 0:1],
                    scalar1=rstd[:],
                    scalar2=-1.0,
                    op0=AluOp.mult,
                    op1=AluOp.mult,
                )
                # normalize + pack: pk[p, c, h*96+f] = gt[p, h//2, h%2, c, f]*rstd - mean*rstd
                nc.scalar.activation(
                    out=pk[:, c].rearrange("p (a b f) -> p a b f", a=N_PAIRS, b=2),
                    in_=gt[:, :, :, c, 0:SHARD],
                    func=Act.Identity,
                    bias=beta[:],
                    scale=rstd[:],
                )
                if affine:
                    nc.vector.tensor_mul(pk[:, c], pk[:, c], sc_sb[:])
                    nc.vector.tensor_add(pk[:, c], pk[:, c], bi_sb[:])
                if c % 2 == 1:
                    # store per pair of chunks so output DMA overlaps compute
                    # instead of bunching at group end
                    dst = bass.AP(
                        out_d,
                        (g * GROUP + (c - 1) * CHUNK) * HIDDEN,
                        [[HIDDEN, CHUNK], [CHUNK * HIDDEN, 2], [1, HIDDEN]],
                    )
                    nc.sync.dma_start(out=dst, in_=pk[:, c - 1 : c + 1, :])

    nc.compile()
    return nc


_kernel_cache: dict = {}
last_results = None


def _get_nc(tok_per_core: int, affine: bool):
    key = (tok_per_core, affine)
    if key not in _kernel_cache:
        _kernel_cache[key] = _build(tok_per_core, affine)
    return _kernel_cache[key]


def _prep_inputs(input_ids, tables, ln_scale, ln_bias):
    input_ids = np.asarray(input_ids)
    tables = np.asarray(tables, dtype=np.float32)
    ln_scale = np.asarray(ln_scale, dtype=np.float32)
    ln_bias = np.asarray(ln_bias, dtype=np.float32)
    B, S = input_ids.shape
    tok_per_core = B * S // N_CORES
    affine = not (np.all(ln_scale == 1.0) and np.all(ln_bias == 0.0))

    tabp = np.zeros((NUM_HASHES, NUM_BUCKETS, PAD), np.float32)
    tabp[:, :, :SHARD] = tables
    tabp = np.ascontiguousarray(tabp.reshape(NUM_HASHES * NUM_BUCKETS, PAD))

    ids_flat = input_ids.reshape(-1).astype(np.int64).astype(np.int32)
    in_maps = []
    for c in range(N_CORES):
        idc = ids_flat[c * tok_per_core : (c + 1) * tok_per_core]
        # wrapped-16 layout: w16[p, g, s] = idc[g*GROUP + s*16 + p], replicated
        # over the 8 gpsimd-core partition groups
        w16 = idc.reshape(-1, GROUP // 16, 16).transpose(2, 0, 1)  # [16, g, s]
        w = np.tile(w16, (8, 1, 1)).reshape(128, -1)
        m = {"ids": np.ascontiguousarray(w), "tables": tabp}
        if affine:
            m["ln_scale"] = np.ascontiguousarray(
                np.broadcast_to(ln_scale[None], (128, HIDDEN))
            )
            m["ln_bias"] = np.ascontiguousarray(
                np.broadcast_to(ln_bias[None], (128, HIDDEN))
            )
        in_maps.append(m)
    return in_maps, tok_per_core, affine, (B, S)


def kernel(input_ids, tables, ln_scale, ln_bias):
    global last_results
    in_maps, tok_per_core, affine, (B, S) = _prep_inputs(
        input_ids, tables, ln_scale, ln_bias
    )
    nc = _get_nc(tok_per_core, affine)
    res = run_bass_kernel_spmd(nc, in_maps, core_ids=list(range(N_CORES)))
    last_results = res
    out = np.stack([r["out"] for r in res.results], axis=0)
    return out.reshape(B, S, HIDDEN)


# revision 22
# speedup vs baseline: 2.6460x; 1.0068x over previous
"""CanineEmbeddings (multi-hash bucket embedding lookup + LayerNorm) on 8 TRN2 cores.

Strategy (data-parallel over batch):
  - Each of the 8 cores handles one batch row (8192 tokens).
  - Hash tables (8 x 16384 x 96 f32) are padded host-side to 128 floats/row
    (512B, zero pad) and replicated to every core in DRAM.
  - Tables are processed in PAIRS so a gathered row index
    (pair_half * 16384 + hash) fits in int16 (max 32767) -> 4 dma_gather
    calls per 1024-token group instead of 8.
  - Hashes ((id+1)*prime & 16383) are computed on-device with DVE int ops.
  - dma_gather (SWDGE) fetches 512B rows; gathered layout is
    [128 tokens(part), pair, half, chunk, 128f].
  - LayerNorm per token via bn_stats/bn_aggr on DVE (exact: pad columns are
    excluded via strided 0:96 access patterns), apply via ACT
    Identity(x*rstd - mean*rstd), which also packs 8x96 shards into
    contiguous 768-float rows.
  - Packed rows are DMA'd straight to the output (3072B descriptors).
"""

import contextlib
import ctypes
import os
import sys
import types

import numpy as np

import concourse.bacc as bacc
import concourse.bass as bass
import concourse.mybir as mybir
import concourse.tile as tile
from concourse.bass_utils import run_bass_kernel_spmd
from concourse.library_config import mlp as _mlp_lib
from concourse.tile import add_dep_helper


def _ensure_axon_ntff_hook():
    """The agent image's ``antenv`` lacks ``axon_hooks``; provide it (and the
    ctypes NTFF profile hook) so run_bass_kernel_spmd(trace=True) works.
    Degrades to a None hook (no trace, run still works) on any failure."""
    if "antenv.axon_hooks" in sys.modules:
        return
    hook = None
    try:
        so_path = "/opt/axon/libaxon_pjrt.so"
        lib = ctypes.CDLL(so_path)
        if hasattr(lib, "axon_start_nrt_profile"):
            lib.axon_start_nrt_profile.argtypes = [
                ctypes.POINTER(ctypes.c_int64),
                ctypes.c_size_t,
            ]
            lib.axon_start_nrt_profile.restype = ctypes.c_int64
            lib.axon_stop_nrt_profile.argtypes = [ctypes.c_char_p]
            lib.axon_stop_nrt_profile.restype = ctypes.c_int64

            @contextlib.contextmanager
            def _hook(output_dir, device_ids):
                import jax

                jax.devices()
                if device_ids:
                    ids = (ctypes.c_int64 * len(device_ids))(*device_ids)
                    rc = lib.axon_start_nrt_profile(ids, len(device_ids))
                else:
                    rc = lib.axon_start_nrt_profile(None, 0)
                if rc != 0:
                    raise RuntimeError(f"axon_start_nrt_profile rc={rc}")
                try:
                    yield
                finally:
                    n = lib.axon_stop_nrt_profile(str(output_dir).encode())
                    print(f"ntff profile: {n} file(s) -> {output_dir}", file=sys.stderr)

            hook = _hook
    except Exception as e:  # pragma: no cover
        print(f"ntff hook unavailable: {e}", file=sys.stderr)
    mod = types.ModuleType("antenv.axon_hooks")
    mod.get_axon_ntff_profile_hook = lambda: hook
    mod.set_axon_ntff_profile_hook = lambda h: None
    sys.modules["antenv.axon_hooks"] = mod


_ensure_axon_ntff_hook()

PRIMES = [31, 43, 59, 61, 73, 97, 103, 113]
NUM_HASHES = 8
NUM_BUCKETS = 16384
HIDDEN = 768
SHARD = 96
PAD = 128  # padded floats per table row (512 bytes)
LN_EPS = 1e-6
N_CORES = 8
GROUP = 1024  # tokens per gather group
CHUNK = 128  # tokens per LayerNorm chunk (one partition sweep)
N_PAIRS = NUM_HASHES // 2

AluOp = mybir.AluOpType
Act = mybir.ActivationFunctionType


def _bn_stats_flat(nc, out, in_):
    """BNStats over the whole (possibly multi-dim) input stream, emitting one
    6-tuple per partition. The bass wrapper asserts per-inner-vector grouping,
    but HW (walrus: 'Output must be 6 elements/partition') accumulates over
    the full stream; with an even inner size the even/odd split stays aligned."""
    assert in_.free_size() <= nc.vector.BN_STATS_FMAX
    assert out.free_size() == 6
    return nc.vector.add_instruction(
        mybir.InstBNStats(
            name=nc.get_next_instruction_name(),
            ins=[nc.vector.lower_ap(in_)],
            outs=[nc.vector.lower_ap(out)],
        )
    )


def _build(tok_per_core: int, affine: bool, enable_asserts: bool = False):
    n_groups = tok_per_core // GROUP
    n_chunks = GROUP // CHUNK  # 8
    wrap_s = GROUP // 16  # 64
    f32, i32, i16 = mybir.dt.float32, mybir.dt.int32, mybir.dt.int16

    nc = bacc.Bacc(
        "TRN2",
        target_bir_lowering=False,
        debug=False,
        enable_asserts=enable_asserts,
        # dma_gather desc-gen runs on the Q7 cpu pair selected by queue_num;
        # 4 queues let up to 4 gathers generate descriptors concurrently.
        num_swdge_queues=4,
    )

    ids_d = nc.dram_tensor("ids", [128, n_groups * wrap_s], i32, kind="ExternalInput")
    tab_d = nc.dram_tensor(
        "tables", [NUM_HASHES * NUM_BUCKETS, PAD], f32, kind="ExternalInput"
    )
    out_d = nc.dram_tensor("out", [tok_per_core, HIDDEN], f32, kind="ExternalOutput")
    if affine:
        sc_d = nc.dram_tensor("ln_scale", [128, HIDDEN], f32, kind="ExternalInput")
        bi_d = nc.dram_tensor("ln_bias", [128, HIDDEN], f32, kind="ExternalInput")

    from contextlib import ExitStack

    with tile.TileContext(nc) as tc, ExitStack() as ctx:
        const = ctx.enter_context(tc.tile_pool(name="const", bufs=1))
        gpool = ctx.enter_context(tc.tile_pool(name="gather", bufs=3))
        ppool = ctx.enter_context(tc.tile_pool(name="packed", bufs=2))
        spool = ctx.enter_context(tc.tile_pool(name="stats", bufs=8))
        tpool = ctx.enter_context(tc.tile_pool(name="tmp", bufs=2))

        # dma_gather is a Q7 extended instruction living in the 'mlp' ucode
        # library; it must be loaded on the Pool engine before any gather.
        lib_inst = nc.gpsimd.load_library(_mlp_lib).ins

        eps_sb = const.tile([128, 1], f32)
        nc.vector.memset(eps_sb[:], LN_EPS)

        ids_sb = const.tile([128, n_groups, wrap_s], i32)
        nc.sync.dma_start(
            out=ids_sb[:],
            in_=ids_d[:].rearrange("p (g s) -> p g s", g=n_groups),
        )
        if affine:
            sc_sb = const.tile([128, HIDDEN], f32)
            nc.sync.dma_start(out=sc_sb[:], in_=sc_d[:])
            bi_sb = const.tile([128, HIDDEN], f32)
            nc.sync.dma_start(out=bi_sb[:], in_=bi_d[:])

        # idx_all[p, g, pair, half, s] = half*16384 + hash_{2*pair+half}(ids[g, s*16+p%16])
        # DVE arithmetic runs in fp32, so keep every intermediate < 2^24:
        # m = (ids & 16383) + 1 <= 16384; t = m*prime <= 1.85e6; idx = (t & 16383) + half*16384.
        idx_all = const.tile([128, n_groups, N_PAIRS, 2, wrap_s], i16)
        m_sb = const.tile([128, n_groups, wrap_s], i32)
        nc.vector.tensor_scalar(
            out=m_sb[:],
            in0=ids_sb[:],
            scalar1=NUM_BUCKETS - 1,
            scalar2=None,
            op0=AluOp.bitwise_and,
        )
        for h in range(NUM_HASHES):
            # (m+1)*p = m*p + p (all-arith); then (t & 16383) | half*16384
            # (all-bitwise) -- walrus rejects mixed bitwise/arith chains.
            tmp = tpool.tile([128, n_groups, wrap_s], i32)
            nc.vector.tensor_scalar(
                out=tmp[:],
                in0=m_sb[:],
                scalar1=PRIMES[h],
                scalar2=PRIMES[h],
                op0=AluOp.mult,
                op1=AluOp.add,
            )
            tmp2 = tpool.tile([128, n_groups, wrap_s], i32)
            nc.vector.tensor_scalar(
                out=tmp2[:],
                in0=tmp[:],
                scalar1=NUM_BUCKETS - 1,
                scalar2=None,
                op0=AluOp.bitwise_and,
            )
            nc.vector.tensor_copy(
                out=idx_all[:, :, h // 2, h % 2, :], in_=tmp2[:]
            )

        for g in range(n_groups):
            # gathered rows: gt[p, pair, half, chunk, f]
            gt = gpool.tile([128, N_PAIRS, 2, n_chunks, PAD], f32)
            # one gather per hash table; num_idxs is capped by the SWDGE
            # descriptor ring (dynamic_dma_scratch_size/16 = 1024 descs)
            for h in range(NUM_HASHES):
                gi = nc.gpsimd.dma_gather(
                    out_ap=gt[:, h // 2, h % 2],
                    in_ap=tab_d[h * NUM_BUCKETS : (h + 1) * NUM_BUCKETS, :],
                    idxs_ap=idx_all[:, g, h // 2, h % 2, :],
                    num_idxs=GROUP,
                    num_idxs_reg=GROUP,
                    elem_size=PAD,
                    queue_num=h % 4,
                )
                add_dep_helper(gi.ins, lib_inst, sync=False, reason="needs mlp lib")
            pk = ppool.tile([128, n_chunks, HIDDEN], f32)
            for c in range(n_chunks):
                # HW BNStats emits one 6-tuple over the whole input stream;
                # 384 elems (4 shards) per call stays under the 512 cap, and
                # the even/odd accumulator split stays row-aligned since 96
                # is even.
                stats = spool.tile([128, 2, 6], f32)
                _bn_stats_flat(nc, stats[:, 0, :], gt[:, 0:2, :, c, 0:SHARD])
                _bn_stats_flat(nc, stats[:, 1, :], gt[:, 2:4, :, c, 0:SHARD])
                mv = spool.tile([128, 2], f32)
                nc.vector.bn_aggr(out=mv[:], in_=stats[:])
                sd = spool.tile([128, 1], f32)
                nc.scalar.activation(
                    out=sd[:], in_=mv[:, 1:2], func=Act.Sqrt, bias=eps_sb[:]
                )
                rstd = spool.tile([128, 1], f32)
                nc.vector.reciprocal(out=rstd[:], in_=sd[:])
                beta = spool.tile([128, 1], f32)
                nc.vector.tensor_scalar(
                    out=beta[:],
                    in0=mv[:, 0:1],
                    scalar1=rstd[:],
                    scalar2=-1.0,
                    op0=AluOp.mult,
                    op1=AluOp.mult,
                )
                # normalize + pack: pk[p, c, h*96+f] = gt[p, h//2, h%2, c, f]*rstd - mean*rstd
                nc.scalar.activation(
                    out=pk[:, c].rearrange("p (a b f) -> p a b f", a=N_PAIRS, b=2),
                    in_=gt[:, :, :, c, 0:SHARD],
                    func=Act.Identity,
                    bias=beta[:],
                    scale=rstd[:],
                )
                if affine:
                    nc.vector.tensor_mul(pk[:, c], pk[:, c], sc_sb[:])
                    nc.vector.tensor_add(pk[:, c], pk[:, c], bi_sb[:])
                if c % 2 == 1:
                    # store per pair of chunks so output DMA overlaps compute
                    # instead of bunching at group end
                    dst = bass.AP(
                        out_d,
                        (g * GROUP + (c - 1) * CHUNK) * HIDDEN,
                        [[HIDDEN, CHUNK], [CHUNK * HIDDEN, 2], [1, HIDDEN]],
                    )
                    nc.sync.dma_start(out=dst, in_=pk[:, c - 1 : c + 1, :])

    nc.compile()
    return nc


_kernel_cache: dict = {}
last_results = None


def _get_nc(tok_per_core: int, affine: bool):
    key = (tok_per_core, affine)
    if key not in _kernel_cache:
        _kernel_cache[key] = _build(tok_per_core, affine)
    return _kernel_cache[key]


def _prep_inputs(input_ids, tables, ln_scale, ln_bias):
    input_ids = np.asarray(input_ids)
    tables = np.asarray(tables, dtype=np.float32)
    ln_scale = np.asarray(ln_scale, dtype=np.float32)
    ln_bias = np.asarray(ln_bias, dtype=np.float32)
    B, S = input_ids.shape
    tok_per_core = B * S // N_CORES
    affine = not (np.all(ln_scale == 1.0) and np.all(ln_bias == 0.0))

    tabp = np.zeros((NUM_HASHES, NUM_BUCKETS, PAD), np.float32)
    tabp[:, :, :SHARD] = tables
    tabp = np.ascontiguousarray(tabp.reshape(NUM_HASHES * NUM_BUCKETS, PAD))

    ids_flat = input_ids.reshape(-1).astype(np.int64).astype(np.int32)
    in_maps = []
    for c in range(N_CORES):
        idc = ids_flat[c * tok_per_core : (c + 1) * tok_per_core]
        # wrapped-16 layout: w16[p, g, s] = idc[g*GROUP + s*16 + p], replicated
        # over the 8 gpsimd-core partition groups
        w16 = idc.reshape(-1, GROUP // 16, 16).transpose(2, 0, 1)  # [16, g, s]
        w = np.tile(w16, (8, 1, 1)).reshape(128, -1)
        m = {"ids": np.ascontiguousarray(w), "tables": tabp}
        if affine:
            m["ln_scale"] = np.ascontiguousarray(
                np.broadcast_to(ln_scale[None], (128, HIDDEN))
            )
            m["ln_bias"] = np.ascontiguousarray(
                np.broadcast_to(ln_bias[None], (128, HIDDEN))
            )
        in_maps.append(m)
    return in_maps, tok_per_core, affine, (B, S)


def kernel(input_ids, tables, ln_scale, ln_bias):
    global last_results
    in_maps, tok_per_core, affine, (B, S) = _prep_inputs(
        input_ids, tables, ln_scale, ln_bias
    )
    nc = _get_nc(tok_per_core, affine)
    res = run_bass_kernel_spmd(nc, in_maps, core_ids=list(range(N_CORES)))
    last_results = res
    out = np.stack([r["out"] for r in res.results], axis=0)
    return out.reshape(B, S, HIDDEN)
